# revision 9
# baseline (speedup 1.0000x reference)
"""Quantized int8 3x3 conv (dequant -> conv -> requant) on 8 TRN2 NeuronCores.

Sharding: data-parallel over batch (16 images -> 2 per core), weights/bias
replicated. No cross-core communication.

v2: the device kernel is pure matmul + requant + big contiguous DMAs.
All layout shuffling happens on the host:
  - input is pre-packed to bf16 (x - IN_ZP, exact for 8-bit ints) in the
    exact SBUF parity layout [group, par*64+c, (blk, img, w)], so each
    group load is one DMA with 4 KiB contiguous per partition.
  - the six lhsT weight tiles and the bias are prebuilt on the host and
    land in one 1.5 KiB/partition DMA (the v1 on-chip weight prep emitted
    ~74k 4-byte gather descriptors that saturated the DMA engines).
  - output is stored in the packed psum layout [unit, (r,o), (q, img, w)]
    as int16 (values are < 2^15 in magnitude), 4 KiB contiguous per
    partition per unit; the host unshuffles to NCHW and widens to int32.

Math (exact-integer in disguise): (x-7) and (w-3) are 8/9-bit ints, exact
in bf16; products < 2^16 and psum partial sums < 2^24 are exact in fp32.
conv is 6 matmuls per output row pair (K = parity*64+c = 128,
M = r*64+o = 128, N = 512 = 2 images x 256 cols interleaved); kw taps are
free-dim shifts.  Requant = one DVE tensor_scalar per pair:
int16(round(psum * 1e-4 + bias)).
"""

import numpy as np
import ml_dtypes

import concourse.bass as bass
import concourse.tile as tile
from concourse import bacc, mybir
from concourse.bass_utils import run_bass_kernel_spmd

N_CORES = 8
IN_ZP = 7
W_ZP = 3
SCALE = 1e-4  # IN_SCALE * W_SCALE; OUT_SCALE=1, OUT_ZP=0, B_SCALE=1, B_ZP=0
BF16 = ml_dtypes.bfloat16

H = W = 256
C = CO = 64
OH = OW = H - 2
N_IMG = 2                 # images per core
BLK = N_IMG * W           # 512 free-dim columns per row-pair block
N_PAIRS = OH // 2         # 127 output row pairs
N_GROUPS = H // 8         # 32 input groups of 4 parity blocks
N_UNITS = (N_PAIRS + 3) // 4  # 32 compute units of up to 4 pairs
GCOL = 4 * BLK            # 2048


def build_nc(n_cores=N_CORES):
    nc = bacc.Bacc("TRN2", target_bir_lowering=False, debug=False,
                   num_devices=n_cores)
    xp = nc.declare_dram_parameter("xp", [N_GROUPS, 128, GCOL],
                                   mybir.dt.bfloat16, isOutput=False)
    wp = nc.declare_dram_parameter("wp", [128, 6 * 128], mybir.dt.bfloat16,
                                   isOutput=False)
    bp = nc.declare_dram_parameter("bp", [128, 1], mybir.dt.float32,
                                   isOutput=False)
    yp = nc.declare_dram_parameter("yp", [N_UNITS, 128, GCOL],
                                   mybir.dt.int16, isOutput=True)

    with tile.TileContext(nc) as tc:
        with (
            tc.tile_pool(name="const", bufs=1) as constp,
            tc.tile_pool(name="x2", bufs=6) as x2p,
            tc.tile_pool(name="outp", bufs=4) as outp,
            tc.tile_pool(name="psum", bufs=8, space="PSUM") as psp,
        ):
            x2_tiles = {}

            def load_group(g, eng=None):
                t = x2p.tile([128, GCOL], mybir.dt.bfloat16, tag="x2")
                (eng or nc.scalar).dma_start(t[:], xp[g])
                x2_tiles[g] = t

            # HAM pre-warm: ~3.4us of dummy PE activity while the first
            # input DMAs are still in flight, so the real matmul stream
            # starts at the full 2.4 GHz clock instead of 1.2
            warm = constp.tile([128, 512], mybir.dt.bfloat16, tag="warm")
            nc.vector.memset(warm[:], 0.0)
            warm_ps = psp.tile([128, BLK], mybir.dt.float32, tag="ps",
                               name="warm_ps")
            for _ in range(8):
                nc.tensor.matmul(warm_ps[:], warm[:, 0:128], warm[:],
                                 start=True, stop=True)

            # prologue: tiny weight/bias loads drain first on each HWDGE
            # queue, then the first group load is split across both queues
            wt = constp.tile([128, 6 * 128], mybir.dt.bfloat16, tag="wt")
            nc.scalar.dma_start(wt[:], wp[:])
            bias_f = constp.tile([128, 1], mybir.dt.float32, tag="bias_f")
            nc.sync.dma_start(bias_f[:], bp[:])
            lhs = [wt[:, t * 128:(t + 1) * 128] for t in range(6)]

            g0 = x2p.tile([128, GCOL], mybir.dt.bfloat16, tag="x2")
            nc.scalar.dma_start(g0[:, 0:GCOL // 2], xp[0][:, 0:GCOL // 2])
            nc.sync.dma_start(g0[:, GCOL // 2:], xp[0][:, GCOL // 2:])
            x2_tiles[0] = g0
            load_group(1, nc.scalar)

            def compute_unit(pairs):
                nq = len(pairs)
                ps = [psp.tile([128, BLK], mybir.dt.float32, tag="ps",
                               name=f"ps_{pairs[0]}_{i}")
                      for i in range(nq)]
                for j2 in range(2):
                    for kw in range(3):
                        lt = lhs[j2 * 3 + kw]
                        # explicit ldweights once per tile: walrus pairs it
                        # with the following matmuls (non-self-loading),
                        # skipping the redundant per-matmul weight reload
                        nc.tensor.ldweights(lt)
                        first = (j2 == 0 and kw == 0)
                        # the start matmul writes the full 512-wide bank;
                        # accumulating taps only need the 510 useful cols
                        width = BLK if first else BLK - 2
                        for q, pair in enumerate(pairs):
                            g2, lb = divmod(pair + j2, 4)
                            rhs = x2_tiles[g2][:, lb * BLK + kw:
                                               lb * BLK + kw + width]
                            nc.tensor.matmul(
                                ps[q][:, 0:width], lt, rhs,
                                start=first,
                                stop=(j2 == 1 and kw == 2))
                ot = outp.tile([128, GCOL], mybir.dt.int16, tag="out")
                for q in range(nq):
                    nc.vector.tensor_scalar(
                        ot[:, q * BLK:(q + 1) * BLK], ps[q][:],
                        SCALE, bias_f[:],
                        mybir.AluOpType.mult, mybir.AluOpType.add)
                u0, q0 = divmod(pairs[0], 4)
                nc.sync.dma_start(yp[u0][:, q0 * BLK:(q0 + nq) * BLK],
                                  ot[:, 0:nq * BLK])

            # 30 units of 4 pairs, then a 4/2/1 tail so the post-stream
            # drain (requant + store of the final unit) is minimal
            unit_pairs = [list(range(4 * u, 4 * u + 4)) for u in range(31)]
            unit_pairs += [[124, 125], [126]]
            for i, pairs in enumerate(unit_pairs):
                compute_unit(pairs)
                if i + 2 < N_GROUPS:
                    load_group(i + 2)

    nc.compile()
    return nc


_NC_CACHE = {}


def get_nc(*_args, **_kwargs):
    if "nc" not in _NC_CACHE:
        _NC_CACHE["nc"] = build_nc()
    return _NC_CACHE["nc"]


def pack_inputs(input, weight, bias):
    """Host-side prepack: returns per-core in_maps."""
    x = np.ascontiguousarray(input, dtype=np.int32)
    # [core, img, ch, g, b, par, w] -> [core, g, par, ch, b, img, w]
    xr = (x.astype(np.int16) - IN_ZP).astype(BF16)
    xr = xr.reshape(N_CORES, N_IMG, C, N_GROUPS, 4, 2, W)
    xr = np.ascontiguousarray(xr.transpose(0, 3, 5, 2, 4, 1, 6))
    xp = xr.reshape(N_CORES, N_GROUPS, 128, GCOL)

    wf = weight.astype(np.float32) - W_ZP            # [O, I, kh, kw]
    lhs = np.zeros((6, 128, 128), np.float32)
    for j2 in range(2):
        for kw in range(3):
            t = j2 * 3 + kw
            for par in range(2):
                for r in range(2):
                    kh = 2 * j2 + par - r
                    if 0 <= kh <= 2:
                        # lhs[t][par*64+c][r*64+o] = wf[o, c, kh, kw]
                        lhs[t, par * 64:par * 64 + 64,
                            r * 64:r * 64 + 64] = wf[:, :, kh, kw].T
    wpk = np.ascontiguousarray(
        lhs.transpose(1, 0, 2)).reshape(128, 6 * 128).astype(BF16)

    bpk = np.concatenate([bias, bias]).astype(np.float32).reshape(128, 1)

    return [{"xp": np.ascontiguousarray(xp[i]), "wp": wpk, "bp": bpk}
            for i in range(N_CORES)]


def unpack_output(yp):
    """[N_UNITS, 128, GCOL] int16 -> [N_IMG, CO, OH, OW] int32."""
    a = yp.reshape(N_UNITS, 2, CO, 4, N_IMG, W)      # [u, r, o, q, img, w]
    a = a.transpose(4, 2, 0, 3, 1, 5)                # [img, o, u, q, r, w]
    a = a.reshape(N_IMG, CO, N_UNITS * 8, W)
    return a[:, :, :OH, :OW].astype(np.int32)


def run_sharded(nc, input, weight, bias, n_img=N_IMG, **kwargs):
    in_maps = pack_inputs(input, weight, bias)
    res = run_bass_kernel_spmd(nc, in_maps, list(range(N_CORES)), **kwargs)
    out = np.concatenate([unpack_output(r["yp"]) for r in res.results],
                         axis=0)
    return out, res


def kernel(input, weight, bias):
    nc = get_nc()
    out, _ = run_sharded(nc, input, weight, bias)
    return out


# revision 13
# speedup vs baseline: 1.1573x; 1.1573x over previous
"""Quantized int8 3x3 conv (dequant -> conv -> requant) on 8 TRN2 NeuronCores.

Sharding: data-parallel over batch (16 images -> 2 per core), weights/bias
replicated. No cross-core communication.

v2: the device kernel is pure matmul + requant + big contiguous DMAs.
All layout shuffling happens on the host:
  - input is pre-packed to bf16 (x - IN_ZP, exact for 8-bit ints) in the
    exact SBUF parity layout [group, par*64+c, (blk, img, w)], so each
    group load is one DMA with 4 KiB contiguous per partition.
  - the six lhsT weight tiles and the bias are prebuilt on the host and
    land in one 1.5 KiB/partition DMA (the v1 on-chip weight prep emitted
    ~74k 4-byte gather descriptors that saturated the DMA engines).
  - output is stored in the packed psum layout [unit, (r,o), (q, img, w)]
    as int16 (values are < 2^15 in magnitude), 4 KiB contiguous per
    partition per unit; the host unshuffles to NCHW and widens to int32.

Math (exact-integer in disguise): (x-7) and (w-3) are 8/9-bit ints, exact
in bf16; products < 2^16 and psum partial sums < 2^24 are exact in fp32.
conv is 6 matmuls per output row pair (K = parity*64+c = 128,
M = r*64+o = 128, N = 512 = 2 images x 256 cols interleaved); kw taps are
free-dim shifts.  Requant = one DVE tensor_scalar per pair:
int16(round(psum * 1e-4 + bias)).
"""

import numpy as np
import ml_dtypes

import concourse.bass as bass
import concourse.tile as tile
from concourse import bacc, mybir
from concourse.bass_utils import run_bass_kernel_spmd

N_CORES = 8
IN_ZP = 7
W_ZP = 3
SCALE = 1e-4  # IN_SCALE * W_SCALE; OUT_SCALE=1, OUT_ZP=0, B_SCALE=1, B_ZP=0
BF16 = ml_dtypes.bfloat16

H = W = 256
C = CO = 64
OH = OW = H - 2
N_IMG = 2                 # images per core
BLK = N_IMG * W           # 512 free-dim columns per row-pair block
N_PAIRS = OH // 2         # 127 output row pairs
N_GROUPS = H // 8         # 32 input groups of 4 parity blocks
N_UNITS = (N_PAIRS + 3) // 4  # 32 compute units of up to 4 pairs
GCOL = 4 * BLK            # 2048


def build_nc(n_cores=N_CORES):
    nc = bacc.Bacc("TRN2", target_bir_lowering=False, debug=False,
                   num_devices=n_cores)
    xp = nc.declare_dram_parameter("xp", [N_GROUPS, 128, GCOL],
                                   mybir.dt.bfloat16, isOutput=False)
    wp = nc.declare_dram_parameter("wp", [128, 6 * 128], mybir.dt.bfloat16,
                                   isOutput=False)
    bp = nc.declare_dram_parameter("bp", [128, 1], mybir.dt.float32,
                                   isOutput=False)
    yp = nc.declare_dram_parameter("yp", [N_UNITS, 128, GCOL],
                                   mybir.dt.int16, isOutput=True)

    with tile.TileContext(nc) as tc:
        with (
            tc.tile_pool(name="const", bufs=1) as constp,
            tc.tile_pool(name="x2", bufs=6) as x2p,
            tc.tile_pool(name="xblk", bufs=8) as xblkp,
            tc.tile_pool(name="outp", bufs=4) as outp,
            tc.tile_pool(name="psum", bufs=8, space="PSUM") as psp,
        ):
            x2_tiles = {}

            def load_group(g, eng=None):
                t = x2p.tile([128, GCOL], mybir.dt.bfloat16, tag="x2")
                (eng or nc.scalar).dma_start(t[:], xp[g])
                x2_tiles[g] = t

            # HAM pre-warm: dummy PE activity bridging the gap between the
            # engine preamble and the first input data landing, so the HAM
            # activity window flips to full clock early
            warm = constp.tile([128, 512], mybir.dt.bfloat16, tag="warm")
            nc.vector.memset(warm[:], 0.0)
            warm_ps = psp.tile([128, BLK], mybir.dt.float32, tag="ps",
                               name="warm_ps")
            for _ in range(4):
                nc.tensor.matmul(warm_ps[:], warm[:, 0:128], warm[:],
                                 start=True, stop=True)

            # prologue: tiny weight/bias loads drain first on each HWDGE
            # queue, then the first two groups load block-by-block (128 KiB
            # each, alternating queues) so the first matmuls are gated on
            # one block, not a whole 512 KiB group
            wt = constp.tile([128, 6 * 128], mybir.dt.bfloat16, tag="wt")
            nc.scalar.dma_start(wt[:], wp[:])
            bias_f = constp.tile([128, 1], mybir.dt.float32, tag="bias_f")
            nc.sync.dma_start(bias_f[:], bp[:])
            lhs = [wt[:, t * 128:(t + 1) * 128] for t in range(6)]

            blk_tiles = []
            for b in range(8):
                t = xblkp.tile([128, BLK], mybir.dt.bfloat16, tag="xb")
                eng = nc.scalar if b % 2 == 0 else nc.sync
                g, lb = divmod(b, 4)
                eng.dma_start(t[:], xp[g][:, lb * BLK:(lb + 1) * BLK])
                blk_tiles.append(t)

            def rhs_slice(g2, lb, off, width):
                if g2 < 2:
                    return blk_tiles[4 * g2 + lb][:, off:off + width]
                return x2_tiles[g2][:, lb * BLK + off:lb * BLK + off + width]

            def compute_unit(pairs):
                nq = len(pairs)
                ps = [psp.tile([128, BLK], mybir.dt.float32, tag="ps",
                               name=f"ps_{pairs[0]}_{i}")
                      for i in range(nq)]
                for j2 in range(2):
                    for kw in range(3):
                        lt = lhs[j2 * 3 + kw]
                        # explicit ldweights once per tile: walrus pairs it
                        # with the following matmuls (non-self-loading),
                        # skipping the redundant per-matmul weight reload
                        nc.tensor.ldweights(lt)
                        first = (j2 == 0 and kw == 0)
                        # the start matmul writes the full 512-wide bank;
                        # accumulating taps only need the 510 useful cols
                        width = BLK if first else BLK - 2
                        for q, pair in enumerate(pairs):
                            g2, lb = divmod(pair + j2, 4)
                            rhs = rhs_slice(g2, lb, kw, width)
                            nc.tensor.matmul(
                                ps[q][:, 0:width], lt, rhs,
                                start=first,
                                stop=(j2 == 1 and kw == 2))
                ot = outp.tile([128, GCOL], mybir.dt.int16, tag="out")
                for q in range(nq):
                    nc.vector.tensor_scalar(
                        ot[:, q * BLK:(q + 1) * BLK], ps[q][:],
                        SCALE, bias_f[:],
                        mybir.AluOpType.mult, mybir.AluOpType.add)
                u0, q0 = divmod(pairs[0], 4)
                nc.sync.dma_start(yp[u0][:, q0 * BLK:(q0 + nq) * BLK],
                                  ot[:, 0:nq * BLK])

            # 30 units of 4 pairs, then a 4/2/1 tail so the post-stream
            # drain (requant + store of the final unit) is minimal
            unit_pairs = [list(range(4 * u, 4 * u + 4)) for u in range(31)]
            unit_pairs += [[124, 125], [126]]
            for i, pairs in enumerate(unit_pairs):
                compute_unit(pairs)
                if i + 2 < N_GROUPS:
                    load_group(i + 2)

    nc.compile()
    return nc


_NC_CACHE = {}


def get_nc(*_args, **_kwargs):
    if "nc" not in _NC_CACHE:
        _NC_CACHE["nc"] = build_nc()
    return _NC_CACHE["nc"]


def pack_inputs(input, weight, bias):
    """Host-side prepack: returns per-core in_maps."""
    x = np.ascontiguousarray(input, dtype=np.int32)
    # [core, img, ch, g, b, par, w] -> [core, g, par, ch, b, img, w]
    xr = (x.astype(np.int16) - IN_ZP).astype(BF16)
    xr = xr.reshape(N_CORES, N_IMG, C, N_GROUPS, 4, 2, W)
    xr = np.ascontiguousarray(xr.transpose(0, 3, 5, 2, 4, 1, 6))
    xp = xr.reshape(N_CORES, N_GROUPS, 128, GCOL)

    wf = weight.astype(np.float32) - W_ZP            # [O, I, kh, kw]
    lhs = np.zeros((6, 128, 128), np.float32)
    for j2 in range(2):
        for kw in range(3):
            t = j2 * 3 + kw
            for par in range(2):
                for r in range(2):
                    kh = 2 * j2 + par - r
                    if 0 <= kh <= 2:
                        # lhs[t][par*64+c][r*64+o] = wf[o, c, kh, kw]
                        lhs[t, par * 64:par * 64 + 64,
                            r * 64:r * 64 + 64] = wf[:, :, kh, kw].T
    wpk = np.ascontiguousarray(
        lhs.transpose(1, 0, 2)).reshape(128, 6 * 128).astype(BF16)

    bpk = np.concatenate([bias, bias]).astype(np.float32).reshape(128, 1)

    return [{"xp": np.ascontiguousarray(xp[i]), "wp": wpk, "bp": bpk}
            for i in range(N_CORES)]


def unpack_output(yp):
    """[N_UNITS, 128, GCOL] int16 -> [N_IMG, CO, OH, OW] int32."""
    a = yp.reshape(N_UNITS, 2, CO, 4, N_IMG, W)      # [u, r, o, q, img, w]
    a = a.transpose(4, 2, 0, 3, 1, 5)                # [img, o, u, q, r, w]
    a = a.reshape(N_IMG, CO, N_UNITS * 8, W)
    return a[:, :, :OH, :OW].astype(np.int32)


def run_sharded(nc, input, weight, bias, n_img=N_IMG, **kwargs):
    in_maps = pack_inputs(input, weight, bias)
    res = run_bass_kernel_spmd(nc, in_maps, list(range(N_CORES)), **kwargs)
    out = np.concatenate([unpack_output(r["yp"]) for r in res.results],
                         axis=0)
    return out, res


def kernel(input, weight, bias):
    nc = get_nc()
    out, _ = run_sharded(nc, input, weight, bias)
    return out


# revision 16
# speedup vs baseline: 1.2221x; 1.0559x over previous
"""Quantized int8 3x3 conv (dequant -> conv -> requant) on 8 TRN2 NeuronCores.

Sharding: data-parallel over batch (16 images -> 2 per core), weights/bias
replicated. No cross-core communication.

v2: the device kernel is pure matmul + requant + big contiguous DMAs.
All layout shuffling happens on the host:
  - input is pre-packed to bf16 (x - IN_ZP, exact for 8-bit ints) in the
    exact SBUF parity layout [group, par*64+c, (blk, img, w)], so each
    group load is one DMA with 4 KiB contiguous per partition.
  - the six lhsT weight tiles and the bias are prebuilt on the host and
    land in one 1.5 KiB/partition DMA (the v1 on-chip weight prep emitted
    ~74k 4-byte gather descriptors that saturated the DMA engines).
  - output is stored in the packed psum layout [unit, (r,o), (q, img, w)]
    as int16 (values are < 2^15 in magnitude), 4 KiB contiguous per
    partition per unit; the host unshuffles to NCHW and widens to int32.

Math (exact-integer in disguise): (x-7) and (w-3) are 8/9-bit ints, exact
in bf16; products < 2^16 and psum partial sums < 2^24 are exact in fp32.
conv is 6 matmuls per output row pair (K = parity*64+c = 128,
M = r*64+o = 128, N = 512 = 2 images x 256 cols interleaved); kw taps are
free-dim shifts.  Requant = one DVE tensor_scalar per pair:
int16(round(psum * 1e-4 + bias)).
"""

import numpy as np
import ml_dtypes

import concourse.bass as bass
import concourse.tile as tile
from concourse import bacc, mybir
from concourse.bass_utils import run_bass_kernel_spmd

N_CORES = 8
IN_ZP = 7
W_ZP = 3
SCALE = 1e-4  # IN_SCALE * W_SCALE; OUT_SCALE=1, OUT_ZP=0, B_SCALE=1, B_ZP=0
BF16 = ml_dtypes.bfloat16

H = W = 256
C = CO = 64
OH = OW = H - 2
N_IMG = 2                 # images per core
BLK = N_IMG * W           # 512 free-dim columns per row-pair block
N_PAIRS = OH // 2         # 127 output row pairs
N_GROUPS = H // 8         # 32 input groups of 4 parity blocks
N_UNITS = (N_PAIRS + 3) // 4  # 32 compute units of up to 4 pairs
GCOL = 4 * BLK            # 2048


def build_nc(n_cores=N_CORES):
    nc = bacc.Bacc("TRN2", target_bir_lowering=False, debug=False,
                   num_devices=n_cores)
    xp = nc.declare_dram_parameter("xp", [N_GROUPS, 128, GCOL],
                                   mybir.dt.bfloat16, isOutput=False)
    wp = nc.declare_dram_parameter("wp", [128, 6 * 128], mybir.dt.bfloat16,
                                   isOutput=False)
    bp = nc.declare_dram_parameter("bp", [128, 1], mybir.dt.float32,
                                   isOutput=False)
    yp = nc.declare_dram_parameter("yp", [N_UNITS, 128, GCOL],
                                   mybir.dt.int16, isOutput=True)

    with tile.TileContext(nc) as tc:
        with (
            tc.tile_pool(name="const", bufs=1) as constp,
            tc.tile_pool(name="x2", bufs=6) as x2p,
            tc.tile_pool(name="outp", bufs=4) as outp,
            tc.tile_pool(name="psum", bufs=8, space="PSUM") as psp,
        ):
            x2_tiles = {}

            def load_group(g, eng=None):
                t = x2p.tile([128, GCOL], mybir.dt.bfloat16, tag="x2")
                (eng or nc.scalar).dma_start(t[:], xp[g])
                x2_tiles[g] = t

            # HAM pre-warm: dummy PE activity bridging the engine preamble
            # and the first input data landing.  The activity window only
            # flips to full clock after a FULLY busy ~3.4us window, so the
            # warmup must run gap-free into the real stream — overshooting
            # the data-ready time slightly is cheaper than undershooting
            # (an idle gap restarts the window and the first ~15 real
            # matmuls run at half clock).
            warm = constp.tile([128, 512], mybir.dt.bfloat16, tag="warm")
            nc.vector.memset(warm[:], 0.0)
            warm_ps = psp.tile([128, BLK], mybir.dt.float32, tag="ps",
                               name="warm_ps")
            for _ in range(10):
                nc.tensor.matmul(warm_ps[:], warm[:, 0:128], warm[:],
                                 start=True, stop=True)

            # prologue: tiny weight/bias loads drain first on each HWDGE
            # queue, then the first group load is split across both queues
            wt = constp.tile([128, 6 * 128], mybir.dt.bfloat16, tag="wt")
            nc.scalar.dma_start(wt[:], wp[:])
            bias_f = constp.tile([128, 1], mybir.dt.float32, tag="bias_f")
            nc.sync.dma_start(bias_f[:], bp[:])
            lhs = [wt[:, t * 128:(t + 1) * 128] for t in range(6)]

            g0 = x2p.tile([128, GCOL], mybir.dt.bfloat16, tag="x2")
            nc.scalar.dma_start(g0[:, 0:GCOL // 2], xp[0][:, 0:GCOL // 2])
            nc.sync.dma_start(g0[:, GCOL // 2:], xp[0][:, GCOL // 2:])
            x2_tiles[0] = g0
            load_group(1, nc.scalar)

            def rhs_slice(g2, lb, off, width):
                return x2_tiles[g2][:, lb * BLK + off:lb * BLK + off + width]

            def compute_unit(pairs):
                nq = len(pairs)
                ps = [psp.tile([128, BLK], mybir.dt.float32, tag="ps",
                               name=f"ps_{pairs[0]}_{i}")
                      for i in range(nq)]
                for j2 in range(2):
                    for kw in range(3):
                        lt = lhs[j2 * 3 + kw]
                        # explicit ldweights once per tile: walrus pairs it
                        # with the following matmuls (non-self-loading),
                        # skipping the redundant per-matmul weight reload
                        nc.tensor.ldweights(lt)
                        first = (j2 == 0 and kw == 0)
                        # the start matmul writes the full 512-wide bank;
                        # accumulating taps only need the 510 useful cols
                        width = BLK if first else BLK - 2
                        for q, pair in enumerate(pairs):
                            g2, lb = divmod(pair + j2, 4)
                            rhs = rhs_slice(g2, lb, kw, width)
                            nc.tensor.matmul(
                                ps[q][:, 0:width], lt, rhs,
                                start=first,
                                stop=(j2 == 1 and kw == 2))
                ot = outp.tile([128, GCOL], mybir.dt.int16, tag="out")
                for q in range(nq):
                    nc.vector.tensor_scalar(
                        ot[:, q * BLK:(q + 1) * BLK], ps[q][:],
                        SCALE, bias_f[:],
                        mybir.AluOpType.mult, mybir.AluOpType.add)
                u0, q0 = divmod(pairs[0], 4)
                # tail units store via the scalar queue (input loads are
                # done by then) so the last stores don't queue behind the
                # big sync-queue backlog
                oeng = nc.scalar if pairs[0] >= 120 else nc.sync
                oeng.dma_start(yp[u0][:, q0 * BLK:(q0 + nq) * BLK],
                               ot[:, 0:nq * BLK])

            # 30 units of 4 pairs, then a 4/2/1 tail so the post-stream
            # drain (requant + store of the final unit) is minimal
            unit_pairs = [list(range(4 * u, 4 * u + 4)) for u in range(31)]
            unit_pairs += [[124, 125], [126]]
            for i, pairs in enumerate(unit_pairs):
                compute_unit(pairs)
                if i + 2 < N_GROUPS:
                    load_group(i + 2)

    nc.compile()
    return nc


_NC_CACHE = {}


def get_nc(*_args, **_kwargs):
    if "nc" not in _NC_CACHE:
        _NC_CACHE["nc"] = build_nc()
    return _NC_CACHE["nc"]


def pack_inputs(input, weight, bias):
    """Host-side prepack: returns per-core in_maps."""
    x = np.ascontiguousarray(input, dtype=np.int32)
    # [core, img, ch, g, b, par, w] -> [core, g, par, ch, b, img, w]
    xr = (x.astype(np.int16) - IN_ZP).astype(BF16)
    xr = xr.reshape(N_CORES, N_IMG, C, N_GROUPS, 4, 2, W)
    xr = np.ascontiguousarray(xr.transpose(0, 3, 5, 2, 4, 1, 6))
    xp = xr.reshape(N_CORES, N_GROUPS, 128, GCOL)

    wf = weight.astype(np.float32) - W_ZP            # [O, I, kh, kw]
    lhs = np.zeros((6, 128, 128), np.float32)
    for j2 in range(2):
        for kw in range(3):
            t = j2 * 3 + kw
            for par in range(2):
                for r in range(2):
                    kh = 2 * j2 + par - r
                    if 0 <= kh <= 2:
                        # lhs[t][par*64+c][r*64+o] = wf[o, c, kh, kw]
                        lhs[t, par * 64:par * 64 + 64,
                            r * 64:r * 64 + 64] = wf[:, :, kh, kw].T
    wpk = np.ascontiguousarray(
        lhs.transpose(1, 0, 2)).reshape(128, 6 * 128).astype(BF16)

    bpk = np.concatenate([bias, bias]).astype(np.float32).reshape(128, 1)

    return [{"xp": np.ascontiguousarray(xp[i]), "wp": wpk, "bp": bpk}
            for i in range(N_CORES)]


def unpack_output(yp):
    """[N_UNITS, 128, GCOL] int16 -> [N_IMG, CO, OH, OW] int32."""
    a = yp.reshape(N_UNITS, 2, CO, 4, N_IMG, W)      # [u, r, o, q, img, w]
    a = a.transpose(4, 2, 0, 3, 1, 5)                # [img, o, u, q, r, w]
    a = a.reshape(N_IMG, CO, N_UNITS * 8, W)
    return a[:, :, :OH, :OW].astype(np.int32)


def run_sharded(nc, input, weight, bias, n_img=N_IMG, **kwargs):
    in_maps = pack_inputs(input, weight, bias)
    res = run_bass_kernel_spmd(nc, in_maps, list(range(N_CORES)), **kwargs)
    out = np.concatenate([unpack_output(r["yp"]) for r in res.results],
                         axis=0)
    return out, res


def kernel(input, weight, bias):
    nc = get_nc()
    out, _ = run_sharded(nc, input, weight, bias)
    return out
